# revision 6
# baseline (speedup 1.0000x reference)
"""GQA causal attention (B=2, S=2048, 32 Q heads / 8 KV heads, d=128) on 8 TRN2 cores.

Sharding: core c -> batch c//4, head-block c%4 (8 Q heads, 2 KV heads each).
Per-core kernel: S^T = K @ Q^T computed directly on the tensor engine so the
PV matmul needs no P transpose; exp without max-subtraction (scores are O(6));
row sums via a ones-column appended to V; causal handled by skipping k>q tiles
plus a triangular mask on diagonal subtiles.
"""

import sys

for p in ("/opt/trn_rl_repo", "/root/.axon_site/_ro/trn_rl_repo"):
    if p not in sys.path:
        sys.path.insert(0, p)

import numpy as np

import concourse.bass as bass
import concourse.mybir as mybir
import concourse.tile as tile
from concourse import bacc
from concourse.bass_utils import run_bass_kernel_spmd
from concourse.masks import make_identity, make_upper_triangular

S = 2048            # sequence length
D = 128             # head size
NQH = 8             # query heads per core
NKVH = 2            # kv heads per core
GROUP = 4           # query heads per kv head
SCALE = 1.0 / np.sqrt(128.0)
NQT = S // D        # 16 q/k tiles of 128 rows
QB = 512            # q-block (4 tiles)
NQB = S // QB       # 4 q-blocks
F32 = mybir.dt.float32


def build_nc():
    nc = bacc.Bacc()
    q_ext = nc.declare_dram_parameter("query", [S, NQH * D], F32, isOutput=False)
    k_ext = nc.declare_dram_parameter("key", [S, NKVH * D], F32, isOutput=False)
    v_ext = nc.declare_dram_parameter("value", [S, NKVH * D], F32, isOutput=False)
    o_ext = nc.declare_dram_parameter("out", [S, NQH * D], F32, isOutput=True)

    with tile.TileContext(nc) as tc:
        with (
            tc.tile_pool(name="singles", bufs=1) as singles,
            tc.tile_pool(name="loads", bufs=4) as loads,
            tc.tile_pool(name="qt", bufs=2) as qt_pool,
            tc.tile_pool(name="pt", bufs=2) as pt_pool,
            tc.tile_pool(name="small", bufs=4) as small,
            tc.tile_pool(name="ost", bufs=4) as ost_pool,
            tc.tile_pool(name="tp", bufs=2, space="PSUM") as tp_pool,
            tc.tile_pool(name="st", bufs=2, space="PSUM") as st_pool,
            tc.tile_pool(name="oa", bufs=2, space="PSUM") as oa_pool,
        ):
            identity = singles.tile([D, D], F32)
            make_identity(nc, identity)
            umask = singles.tile([D, D], F32)  # umask[k, q] = 1.0 if k <= q else 0
            make_upper_triangular(nc, umask, val=1.0, diag=True)

            # dram views tiled as [row_in_tile, tile, head, col]
            kv_view = k_ext[:].rearrange("(ki k) (g d) -> k ki g d", k=D, d=D)
            vv_view = v_ext[:].rearrange("(ki k) (g d) -> k ki g d", k=D, d=D)
            q_view = q_ext[:].rearrange(
                "(qb j r) (h d) -> r qb j h d", r=D, j=QB // D, d=D
            )

            # K^T per kv head: [d, ki, k_off]  (via PE transpose)
            kts = []
            for g in range(NKVH):
                kload = loads.tile([D, NQT, D], F32, tag=f"kload{g}")
                nc.sync.dma_start(out=kload, in_=kv_view[:, :, g, :])
                kt = singles.tile([D, NQT, D], F32, tag=f"kt{g}")
                kts.append(kt)
                for ki in range(NQT):
                    tp = tp_pool.tile([D, D], F32, tag="tp")
                    nc.tensor.transpose(tp, kload[:, ki, :], identity)
                    nc.vector.tensor_copy(kt[:, ki, :], tp)

            # V per kv head, natural layout + ones column: [k_off, ki, d|1]
            vas = []
            for g in range(NKVH):
                va = singles.tile([D, NQT, D + 1], F32, tag=f"va{g}")
                vas.append(va)
                nc.vector.memset(va[:, :, D], 1.0)
                nc.sync.dma_start(out=va[:, :, 0:D], in_=vv_view[:, :, g, :])

            for h in range(NQH):
                g = h // GROUP
                kt = kts[g]
                va = vas[g]
                for qb in range(NQB):
                    nki = 4 * qb + 4  # causal: k tiles 0 .. 4qb+3
                    # Q^T for this 512-row q block
                    qload = loads.tile([D, QB // D, D], F32, tag="qload")
                    nc.sync.dma_start(out=qload, in_=q_view[:, qb, :, h, :])
                    qt = qt_pool.tile([D, QB], F32, tag="qt")
                    for j in range(4):
                        tp = tp_pool.tile([D, D], F32, tag="tp")
                        nc.tensor.transpose(tp, qload[:, j, :], identity)
                        nc.vector.tensor_copy(qt[:, j * D:(j + 1) * D], tp)

                    # S^T chunks (2 k-tiles per chunk) -> exp -> P^T in SBUF
                    pt = pt_pool.tile([D, NQT, QB], F32, tag="pt")
                    for c in range(nki // 2):
                        st = st_pool.tile([D, 2, QB], F32, tag="st")
                        for j in range(2):
                            ki = 2 * c + j
                            nc.tensor.matmul(
                                st[:, j, :], kt[:, ki, :], qt,
                                start=True, stop=True,
                            )
                        nc.scalar.activation(
                            pt[:, 2 * c:2 * c + 2, :], st,
                            mybir.ActivationFunctionType.Exp,
                            scale=float(SCALE),
                        )
                    # causal fixups on the 4 diagonal k-tiles
                    for ki in range(4 * qb, 4 * qb + 4):
                        for qs in range(4):
                            qi = 4 * qb + qs
                            blk = pt[:, ki, qs * D:(qs + 1) * D]
                            if ki > qi:
                                nc.vector.memset(blk, 0.0)
                            elif ki == qi:
                                nc.vector.tensor_mul(blk, blk, umask)

                    # PV with ones column: out_aug[q, 0:128]=O, [:,128]=row sums
                    for qs in range(4):
                        qi = 4 * qb + qs
                        oa = oa_pool.tile([D, D + 1], F32, tag="oa")
                        for ki in range(qi + 1):
                            nc.tensor.matmul(
                                oa, pt[:, ki, qs * D:(qs + 1) * D], va[:, ki, :],
                                start=(ki == 0), stop=(ki == qi),
                            )
                        rl = small.tile([D, 1], F32, tag="rl")
                        nc.vector.reciprocal(rl, oa[:, D:D + 1])
                        ot = ost_pool.tile([D, D], F32, tag="ot")
                        nc.vector.tensor_scalar_mul(ot, oa[:, 0:D], rl)
                        r0 = qb * QB + qs * D
                        nc.sync.dma_start(
                            out=o_ext[r0:r0 + D, h * D:(h + 1) * D], in_=ot
                        )
    nc.finalize()
    return nc


_NC_CACHE = None


def kernel(**inputs):
    global _NC_CACHE
    query = np.asarray(inputs["query"], dtype=np.float32)
    key = np.asarray(inputs["key"], dtype=np.float32)
    value = np.asarray(inputs["value"], dtype=np.float32)

    in_maps = []
    for c in range(8):
        b, hb = c // 4, c % 4
        in_maps.append({
            "query": np.ascontiguousarray(query[b, :, hb * 1024:(hb + 1) * 1024]),
            "key": np.ascontiguousarray(key[b, :, hb * 256:(hb + 1) * 256]),
            "value": np.ascontiguousarray(value[b, :, hb * 256:(hb + 1) * 256]),
        })

    if _NC_CACHE is None:
        _NC_CACHE = build_nc()
    res = run_bass_kernel_spmd(_NC_CACHE, in_maps, list(range(8)))

    out = np.empty((2, 2048, 4096), dtype=np.float32)
    for c in range(8):
        b, hb = c // 4, c % 4
        out[b, :, hb * 1024:(hb + 1) * 1024] = res.results[c]["out"]
    return out


# revision 13
# speedup vs baseline: 2.5179x; 2.5179x over previous
"""GQA causal attention (B=2, S=2048, 32 Q heads / 8 KV heads, d=128) on 8 TRN2 cores.

Sharding: core c -> batch c//4, head-block c%4 (8 Q heads, 2 KV heads each).
Per-core kernel: S^T = K @ Q^T computed directly on the tensor engine so the
PV matmul needs no P transpose; exp without max-subtraction (scores are O(6));
row sums via a ones-column appended to V; causal handled by skipping k>q tiles
plus a triangular mask on diagonal subtiles.
"""

import sys

for p in ("/opt/trn_rl_repo", "/root/.axon_site/_ro/trn_rl_repo"):
    if p not in sys.path:
        sys.path.insert(0, p)

import numpy as np

import concourse.bass as bass
import concourse.mybir as mybir
import concourse.tile as tile
from concourse import bacc
from concourse.bass_utils import run_bass_kernel_spmd
from concourse.masks import make_identity, make_upper_triangular

S = 2048            # sequence length
D = 128             # head size
NQH = 8             # query heads per core
NKVH = 2            # kv heads per core
GROUP = 4           # query heads per kv head
SCALE = 1.0 / np.sqrt(128.0)
NQT = S // D        # 16 q/k tiles of 128 rows
QB = 512            # q-block (4 tiles)
NQB = S // QB       # 4 q-blocks
F32 = mybir.dt.float32
BF16 = mybir.dt.bfloat16


def build_nc():
    nc = bacc.Bacc()
    q_ext = nc.declare_dram_parameter("query", [S, NQH * D], F32, isOutput=False)
    k_ext = nc.declare_dram_parameter("key", [S, NKVH * D], F32, isOutput=False)
    v_ext = nc.declare_dram_parameter("value", [S, NKVH * D], F32, isOutput=False)
    o_ext = nc.declare_dram_parameter("out", [S, NQH * D], F32, isOutput=True)

    with tile.TileContext(nc) as tc:
        with (
            tc.tile_pool(name="singles", bufs=1) as singles,
            tc.tile_pool(name="loads", bufs=4) as loads,
            tc.tile_pool(name="qt", bufs=2) as qt_pool,
            tc.tile_pool(name="pt", bufs=2) as pt_pool,
            tc.tile_pool(name="small", bufs=4) as small,
            tc.tile_pool(name="ost", bufs=4) as ost_pool,
            tc.tile_pool(name="tp", bufs=2, space="PSUM") as tp_pool,
            tc.tile_pool(name="st", bufs=2, space="PSUM") as st_pool,
            tc.tile_pool(name="oa", bufs=2, space="PSUM") as oa_pool,
        ):
            identity = singles.tile([D, D], F32)
            make_identity(nc, identity)
            umask = singles.tile([D, D], BF16)  # umask[k, q] = 1.0 if k <= q else 0
            make_upper_triangular(nc, umask, val=1.0, diag=True)

            # dram views tiled as [row_in_tile, tile, head, col]
            kv_view = k_ext[:].rearrange("(ki k) (g d) -> k ki g d", k=D, d=D)
            vv_view = v_ext[:].rearrange("(ki k) (g d) -> k ki g d", k=D, d=D)
            q_view = q_ext[:].rearrange(
                "(qb j r) (h d) -> r qb j h d", r=D, j=QB // D, d=D
            )

            # K^T per kv head: [d, ki, k_off]  (via PE transpose)
            kts = []
            for g in range(NKVH):
                kload = loads.tile([D, NQT, D], F32, tag=f"kload{g}")
                nc.sync.dma_start(out=kload, in_=kv_view[:, :, g, :])
                kt = singles.tile([D, NQT, D], BF16, tag=f"kt{g}")
                kts.append(kt)
                for ki in range(NQT):
                    tp = tp_pool.tile([D, D], F32, tag="tp")
                    nc.tensor.transpose(tp, kload[:, ki, :], identity)
                    nc.vector.tensor_copy(kt[:, ki, :], tp)

            # V per kv head, natural layout + ones column: [k_off, ki, d|1]
            vas = []
            for g in range(NKVH):
                vload = loads.tile([D, NQT, D], F32, tag=f"vload{g}")
                nc.sync.dma_start(out=vload, in_=vv_view[:, :, g, :])
                va = singles.tile([D, NQT, D + 1], BF16, tag=f"va{g}")
                vas.append(va)
                nc.vector.memset(va[:, :, D], 1.0)
                nc.vector.tensor_copy(va[:, :, 0:D], vload)

            for h in range(NQH):
                g = h // GROUP
                kt = kts[g]
                va = vas[g]
                for qb in range(NQB):
                    nki = 4 * qb + 4  # causal: k tiles 0 .. 4qb+3
                    # Q^T for this 512-row q block
                    qload = loads.tile([D, QB // D, D], F32, tag="qload")
                    nc.sync.dma_start(out=qload, in_=q_view[:, qb, :, h, :])
                    qt = qt_pool.tile([D, QB], BF16, tag="qt")
                    for j in range(4):
                        tp = tp_pool.tile([D, D], F32, tag="tp")
                        nc.tensor.transpose(tp, qload[:, j, :], identity)
                        nc.vector.tensor_copy(qt[:, j * D:(j + 1) * D], tp)

                    # S^T chunks (2 k-tiles per chunk) -> exp -> P^T in SBUF
                    pt = pt_pool.tile([D, NQT, QB], BF16, tag="pt")
                    for c in range(nki // 2):
                        st = st_pool.tile([D, 2, QB], F32, tag="st")
                        for j in range(2):
                            ki = 2 * c + j
                            nc.tensor.matmul(
                                st[:, j, :], kt[:, ki, :], qt,
                                start=True, stop=True,
                            )
                        nc.scalar.activation(
                            pt[:, 2 * c:2 * c + 2, :], st,
                            mybir.ActivationFunctionType.Exp,
                            scale=float(SCALE),
                        )
                    # causal fixup: mask the diagonal blocks (ki == qi).
                    # Blocks with ki > qi are never read by the PV loop.
                    for qs in range(4):
                        qi = 4 * qb + qs
                        blk = pt[:, qi, qs * D:(qs + 1) * D]
                        nc.vector.tensor_mul(blk, blk, umask)

                    # PV with ones column: out_aug[q, 0:128]=O, [:,128]=row sums
                    for qs in range(4):
                        qi = 4 * qb + qs
                        oa = oa_pool.tile([D, D + 1], F32, tag="oa")
                        for ki in range(qi + 1):
                            nc.tensor.matmul(
                                oa, pt[:, ki, qs * D:(qs + 1) * D], va[:, ki, :],
                                start=(ki == 0), stop=(ki == qi),
                            )
                        rl = small.tile([D, 1], F32, tag="rl")
                        nc.vector.reciprocal(rl, oa[:, D:D + 1])
                        ot = ost_pool.tile([D, D], F32, tag="ot")
                        nc.vector.tensor_scalar_mul(ot, oa[:, 0:D], rl)
                        r0 = qb * QB + qs * D
                        nc.sync.dma_start(
                            out=o_ext[r0:r0 + D, h * D:(h + 1) * D], in_=ot
                        )
    nc.finalize()
    return nc


_NC_CACHE = None


def kernel(**inputs):
    global _NC_CACHE
    query = np.asarray(inputs["query"], dtype=np.float32)
    key = np.asarray(inputs["key"], dtype=np.float32)
    value = np.asarray(inputs["value"], dtype=np.float32)

    in_maps = []
    for c in range(8):
        b, hb = c // 4, c % 4
        in_maps.append({
            "query": np.ascontiguousarray(query[b, :, hb * 1024:(hb + 1) * 1024]),
            "key": np.ascontiguousarray(key[b, :, hb * 256:(hb + 1) * 256]),
            "value": np.ascontiguousarray(value[b, :, hb * 256:(hb + 1) * 256]),
        })

    if _NC_CACHE is None:
        _NC_CACHE = build_nc()
    res = run_bass_kernel_spmd(_NC_CACHE, in_maps, list(range(8)))

    out = np.empty((2, 2048, 4096), dtype=np.float32)
    for c in range(8):
        b, hb = c // 4, c % 4
        out[b, :, hb * 1024:(hb + 1) * 1024] = res.results[c]["out"]
    return out
